# revision 44
# baseline (speedup 1.0000x reference)
"""FFT_Net Trainium2 kernel.

Per (batch, channel): Range DFT (512) then Doppler DFT (256) as complex
GEMMs on the TensorEngine in float32r, followed by InstanceNorm fused on
the vector/scalar engines. Data-parallel over the batch dim across 8
NeuronCores.

Key tricks:
- Both DFT stages keep x / y as the *stationary* matmul operand so no
  transposes are ever materialized (stage 1 computes y^T = x^T @ W512,
  stage 2 consumes y^T as lhsT to produce z in natural orientation).
- Stage 1 uses 3-multiplication Karatsuba for the complex GEMM; M3 is
  accumulated with negated weights on top of M2's PSUM bank.
- Stage 2 streams concatenated weights [Wr|Wi] so one accumulation
  group yields [zr | zi] in a single PSUM bank.
- InstanceNorm mean needs no reduction: sum(z) over an instance equals
  512*256*x[0,0] exactly (DFT matrix rows sum to N*delta_0), so the
  mean is the DC element of the input.
- The variance reduction: row-wise sumsq partials on ACT, then one
  GpSimd partition_all_reduce yields partition-replicated stats so the
  normalize scalars need no broadcast.
- Matmul operands are fp16 (PSUM accumulation stays fp32): same 1
  cycle/row as float32r but the weight-load path is 2x faster, which
  otherwise paces the back-to-back matmul stream.
- Per-(b,c) stats/normalize chains are emitted one iteration behind the
  GEMM stream so the TensorEngine never waits on them.

kernel(**inputs) takes the FULL inputs and returns the FULL output.
"""
import sys

sys.path.insert(0, "/opt/trn_rl_repo")

import numpy as np

import concourse.bass as bass  # noqa: F401
import concourse.tile as tile
from concourse import bacc, bass_isa, mybir
from concourse.bass_utils import run_bass_kernel_spmd

B, C, R, D = 16, 16, 512, 256
NCORES = 8
BS = B // NCORES  # batches per core
EPS = 1e-5
N_NORM = R * D
F32 = mybir.dt.float32
F32R = mybir.dt.float32r
F16 = mybir.dt.float16
MULT = mybir.AluOpType.mult
ADD = mybir.AluOpType.add
SUB = mybir.AluOpType.subtract
COPY = mybir.ActivationFunctionType.Copy
SQRT = mybir.ActivationFunctionType.Sqrt
SQUARE = mybir.ActivationFunctionType.Square
X_AXIS = mybir.AxisListType.X


def build():
    nc = bacc.Bacc(None, target_bir_lowering=False)

    xr_d = nc.dram_tensor("x_real", [BS, C, R, D], F16, kind="ExternalInput")
    xi_d = nc.dram_tensor("x_imag", [BS, C, R, D], F16, kind="ExternalInput")
    xpi_d = nc.dram_tensor("x_pi", [BS, C, R, D], F16, kind="ExternalInput")
    wr512_d = nc.dram_tensor("Wr512", [512, 512], F16, kind="ExternalInput")
    wi512_d = nc.dram_tensor("Wi512", [512, 512], F16, kind="ExternalInput")
    nwrpi512_d = nc.dram_tensor("nWrpi512", [512, 512], F16,
                                kind="ExternalInput")
    # catA = [Wr256 | Wi256], catB = [-Wi256 | Wr256]  (both [256, 512])
    w256a_d = nc.dram_tensor("W256catA", [256, 512], F16, kind="ExternalInput")
    w256b_d = nc.dram_tensor("W256catB", [256, 512], F16, kind="ExternalInput")
    out_d = nc.dram_tensor("out", [BS, 2 * C, R, D], F32, kind="ExternalOutput")

    with tile.TileContext(nc) as tc:
        with tc.tile_pool(name="wpool", bufs=1) as wpool, \
             tc.tile_pool(name="xpool", bufs=4) as xpool, \
             tc.tile_pool(name="ypool", bufs=3) as ypool, \
             tc.tile_pool(name="zpool", bufs=4) as zpool, \
             tc.tile_pool(name="stpool", bufs=6) as stpool, \
             tc.tile_pool(name="sqpool", bufs=4) as sqpool, \
             tc.tile_pool(name="drpool", bufs=8, space="DRAM") as drpool, \
             tc.tile_pool(name="pspool", bufs=1, space="PSUM") as pspool:

            # --- weights, resident for the whole kernel ---
            # Spread across rings so the first matmuls aren't queued behind
            # 5MB of weights: wr/nwrpi on the ACT HWDGE ring, wi + w256 on
            # the SWDGE ring (x tiles own the SP ring).
            w512 = {}
            w256 = {}
            for nm, dram, shape, eng, store in (
                    ("wr", wr512_d, [128, 4, 512], nc.scalar, w512),
                    ("wi", wi512_d, [128, 4, 512], nc.gpsimd, w512),
                    ("nwrpi", nwrpi512_d, [128, 4, 512], nc.scalar, w512),
                    ("a", w256a_d, [128, 2, 512], nc.gpsimd, w256),
                    ("b", w256b_d, [128, 2, 512], nc.gpsimd, w256)):
                t = wpool.tile(shape, F16, name=f"w_{nm}")
                eng.dma_start(
                    out=t,
                    in_=dram[:].rearrange("(k p) n -> p k n", p=128))
                store[nm] = t
            eps128 = wpool.tile([128, 1], F32, name="eps128")
            nc.vector.memset(eps128, EPS)

            def emit_compute(b, c):
                """GEMM stream for one (b, c): loads, stage 1, stage 2,
                z PSUM->SBUF copies + sumsq partials. Returns state for the
                deferred stats/normalize pass."""
                xr = xpool.tile([128, 4, 256], F16, name="xr", tag="xr")
                nc.sync.dma_start(
                    out=xr,
                    in_=xr_d[b, c].rearrange("(k p) d -> p k d", p=128))
                xi = xpool.tile([128, 4, 256], F16, name="xi", tag="xi")
                nc.sync.dma_start(
                    out=xi,
                    in_=xi_d[b, c].rearrange("(k p) d -> p k d", p=128))

                xpi = xpool.tile([128, 4, 256], F16, name="xpi", tag="xpi")
                nc.sync.dma_start(
                    out=xpi,
                    in_=xpi_d[b, c].rearrange("(k p) d -> p k d", p=128))

                # --- stage 1 (Karatsuba): yT = (W512 @ x)^T ---
                yT = {}
                asbs = {}
                psB = {}
                for m in range(2):
                    pA = pspool.tile([128, 512], F32, name="ps1a",
                                     tag="ps1a", bufs=2)
                    pB = pspool.tile([128, 512], F32, name="ps1b",
                                     tag="ps1b", bufs=2)
                    psB[m] = pB
                    for k in range(4):
                        nc.tensor.matmul(
                            out=pA, lhsT=xr[:, k, m * 128:(m + 1) * 128],
                            rhs=w512["wr"][:, k, :],
                            start=(k == 0), stop=(k == 3))
                    for k in range(4):
                        nc.tensor.matmul(
                            out=pB, lhsT=xi[:, k, m * 128:(m + 1) * 128],
                            rhs=w512["wi"][:, k, :],
                            start=(k == 0), stop=(k == 3))
                    asb = ypool.tile([128, 512], F32, name=f"asb{m}",
                                     tag=f"asb{m}")
                    nc.vector.tensor_copy(out=asb, in_=pA)
                    asbs[m] = asb
                    bsb = ypool.tile([128, 512], F32, name=f"bsb{m}",
                                     tag=f"bsb{m}")
                    nc.vector.tensor_copy(out=bsb, in_=pB)
                    yt = ypool.tile([128, 512], F16, name=f"yT_r{m}",
                                    tag=f"yT_r{m}")
                    nc.vector.tensor_sub(out=yt, in0=asb, in1=bsb)
                    yT[("r", m)] = yt
                for m in range(2):
                    pB = psB[m]
                    for k in range(4):
                        nc.tensor.matmul(
                            out=pB, lhsT=xpi[:, k, m * 128:(m + 1) * 128],
                            rhs=w512["nwrpi"][:, k, :],
                            start=False, stop=(k == 3))
                    yt = ypool.tile([128, 512], F16, name=f"yT_i{m}",
                                    tag=f"yT_i{m}")
                    # yiT = (-1)*(M2 - M3) - M1
                    nc.vector.scalar_tensor_tensor(
                        out=yt, in0=pB, scalar=-1.0, in1=asbs[m],
                        op0=MULT, op1=SUB)
                    yT[("i", m)] = yt

                # --- stage 2: [zr | zi] = y @ [catA ; catB] ---
                # partials cols: 0-3 q_r (per m2), 4-7 q_i, 8-9 DC mean
                # (mean of the instance == DC input element, exactly)
                partials = stpool.tile([128, 10], F32, name="partials",
                                       tag="partials")
                nc.vector.memset(partials[:, 8:10], 0.0)
                nc.vector.tensor_copy(out=partials[0:1, 8:9],
                                      in_=xr[0:1, 0, 0:1])
                nc.vector.tensor_copy(out=partials[0:1, 9:10],
                                      in_=xi[0:1, 0, 0:1])
                z_r = zpool.tile([128, 4, 256], F32, name="z_r", tag="z_r")
                z_i = zpool.tile([128, 4, 256], F32, name="z_i", tag="z_i")
                zt = {"r": z_r, "i": z_i}
                for m2 in range(4):
                    ps2 = pspool.tile([128, 512], F32, name="ps2",
                                      tag="ps2", bufs=4)
                    n = 0
                    for src_comp, w in (("r", w256["a"]), ("i", w256["b"])):
                        for k2 in range(2):
                            nc.tensor.matmul(
                                out=ps2,
                                lhsT=yT[(src_comp, k2)][
                                    :, m2 * 128:(m2 + 1) * 128],
                                rhs=w[:, k2, :],
                                start=(n == 0), stop=(n == 3))
                            n += 1
                    for ci, comp in enumerate(("r", "i")):
                        half = ps2[:, ci * 256:(ci + 1) * 256]
                        col = ci * 4 + m2
                        sq = sqpool.tile([128, 256], F32, name="sq", tag="sq")
                        if ci == 0:
                            # r half drains via DVE: copy, then fused sumsq
                            nc.vector.tensor_copy(out=zt[comp][:, m2, :],
                                                  in_=half)
                            zsb = zt[comp][:, m2, :]
                            nc.vector.scalar_tensor_tensor(
                                out=sq, in0=zsb, scalar=1.0, in1=zsb,
                                op0=MULT, op1=MULT,
                                accum_out=partials[:, col:col + 1])
                        else:
                            # i half drains via ACT: copy + Square-accumulate
                            nc.scalar.copy(out=zt[comp][:, m2, :], in_=half)
                            nc.scalar.activation(
                                out=sq, in_=half, func=SQUARE,
                                accum_out=partials[:, col:col + 1])
                return dict(b=b, c=c, partials=partials, zt=zt)

            def emit_stats(st):
                """Deferred per-(b,c): cross-partition sumsq reduce (GpSimd,
                replicated to all partitions), stats math at [128,2], then
                normalize + store. No PE or DMA involvement."""
                b, c = st["b"], st["c"]
                partials, zt = st["partials"], st["zt"]
                allred = stpool.tile([128, 10], F32, name="allred",
                                     tag="allred")
                nc.gpsimd.partition_all_reduce(
                    allred, partials, channels=128,
                    reduce_op=bass_isa.ReduceOp.add)
                q2 = stpool.tile([128, 2], F32, name="q2", tag="q2")
                nc.vector.tensor_reduce(
                    out=q2,
                    in_=allred[:, 0:8].rearrange("p (g m) -> p g m", m=4),
                    axis=X_AXIS, op=ADD)
                # var = E[z^2] - mean^2 ; istd = 1/sqrt(var + eps)
                e2 = stpool.tile([128, 2], F32, name="e2", tag="e2")
                nc.vector.tensor_scalar_mul(out=e2, in0=q2,
                                            scalar1=1.0 / N_NORM)
                mean2 = allred[:, 8:10]
                msq = stpool.tile([128, 2], F32, name="msq", tag="msq")
                nc.vector.tensor_mul(out=msq, in0=mean2, in1=mean2)
                var2 = stpool.tile([128, 2], F32, name="var2", tag="var2")
                nc.vector.tensor_sub(out=var2, in0=e2, in1=msq)
                std2 = stpool.tile([128, 2], F32, name="std2", tag="std2")
                nc.scalar.activation(out=std2, in_=var2, func=SQRT,
                                     bias=eps128, scale=1.0)
                istd = stpool.tile([128, 2], F32, name="istd", tag="istd")
                nc.vector.reciprocal(out=istd, in_=std2)
                mb = stpool.tile([128, 2], F32, name="mb", tag="mb")
                nc.vector.tensor_mul(out=mb, in0=mean2, in1=istd)
                # normalize in place and store
                for ci, comp in enumerate(("r", "i")):
                    z = zt[comp]
                    nc.vector.tensor_scalar(
                        out=z, in0=z,
                        scalar1=istd[:, ci:ci + 1],
                        scalar2=mb[:, ci:ci + 1],
                        op0=MULT, op1=SUB)
                    ch = c if comp == "r" else C + c
                    nc.sync.dma_start(
                        out=out_d[b, ch].rearrange("(k p) d -> p k d", p=128),
                        in_=z)

            prev = None
            for b in range(BS):
                for c in range(C):
                    st = emit_compute(b, c)
                    if prev is not None:
                        emit_stats(prev)
                    prev = st
            emit_stats(prev)

    nc.finalize()
    return nc


_NC_CACHE = None


def _get_nc():
    global _NC_CACHE
    if _NC_CACHE is None:
        _NC_CACHE = build()
    return _NC_CACHE


def make_in_maps(inputs):
    xr = np.ascontiguousarray(np.asarray(inputs["x_real"], dtype=np.float32).astype(np.float16))
    xi = np.ascontiguousarray(np.asarray(inputs["x_imag"], dtype=np.float32).astype(np.float16))
    xpi = np.ascontiguousarray(
        (np.asarray(inputs["x_real"], dtype=np.float32)
         + np.asarray(inputs["x_imag"], dtype=np.float32)).astype(np.float16))
    wr512 = np.ascontiguousarray(np.asarray(inputs["Wr512"], dtype=np.float32))
    wi512 = np.ascontiguousarray(np.asarray(inputs["Wi512"], dtype=np.float32))
    wr256 = np.ascontiguousarray(np.asarray(inputs["Wr256"], dtype=np.float32))
    wi256 = np.ascontiguousarray(np.asarray(inputs["Wi256"], dtype=np.float32))
    nwrpi512 = np.ascontiguousarray((-(wr512 + wi512)).astype(np.float16))
    w256a = np.ascontiguousarray(np.concatenate([wr256, wi256], axis=1).astype(np.float16))
    w256b = np.ascontiguousarray(np.concatenate([-wi256, wr256], axis=1).astype(np.float16))
    in_maps = []
    for i in range(NCORES):
        in_maps.append({
            "x_real": np.ascontiguousarray(xr[i * BS:(i + 1) * BS]),
            "x_imag": np.ascontiguousarray(xi[i * BS:(i + 1) * BS]),
            "x_pi": np.ascontiguousarray(xpi[i * BS:(i + 1) * BS]),
            "Wr512": wr512.astype(np.float16), "Wi512": wi512.astype(np.float16), "nWrpi512": nwrpi512,
            "W256catA": w256a, "W256catB": w256b,
        })
    return in_maps


def run(inputs, trace=False):
    nc = _get_nc()
    in_maps = make_in_maps(inputs)
    try:
        res = run_bass_kernel_spmd(nc, in_maps, list(range(NCORES)),
                                   trace=trace)
    except Exception:
        # transient device wedge (NRT_EXEC_UNIT_UNRECOVERABLE): retry once
        res = run_bass_kernel_spmd(nc, in_maps, list(range(NCORES)),
                                   trace=trace)
    out = np.concatenate([res.results[i]["out"] for i in range(NCORES)],
                         axis=0)
    return out, res


def kernel(**inputs):
    out, _ = run(inputs, trace=False)
    return out


if __name__ == "__main__":
    rng = np.random.default_rng(0)
    ins = {
        "x_real": rng.standard_normal((B, C, R, D)).astype(np.float32),
        "x_imag": rng.standard_normal((B, C, R, D)).astype(np.float32),
    }
    n = np.arange(512)
    W = np.exp(-2j * np.pi * np.outer(n, n) / 512).astype(np.complex64)
    ins["Wr512"], ins["Wi512"] = W.real.copy(), W.imag.copy()
    n = np.arange(256)
    W = np.exp(-2j * np.pi * np.outer(n, n) / 256).astype(np.complex64)
    ins["Wr256"], ins["Wi256"] = W.real.copy(), W.imag.copy()
    out = kernel(**ins)
    print("out", out.shape, out.dtype, float(np.abs(out).mean()))


# revision 45
# speedup vs baseline: 1.0916x; 1.0916x over previous
"""FFT_Net Trainium2 kernel.

Per (batch, channel): Range DFT (512) then Doppler DFT (256) as complex
GEMMs on the TensorEngine in float32r, followed by InstanceNorm fused on
the vector/scalar engines. Data-parallel over the batch dim across 8
NeuronCores.

Key tricks:
- Both DFT stages keep x / y as the *stationary* matmul operand so no
  transposes are ever materialized (stage 1 computes y^T = x^T @ W512,
  stage 2 consumes y^T as lhsT to produce z in natural orientation).
- Stage 1 uses 3-multiplication Karatsuba for the complex GEMM; M3 is
  accumulated with negated weights on top of M2's PSUM bank.
- Stage 2 streams concatenated weights [Wr|Wi] so one accumulation
  group yields [zr | zi] in a single PSUM bank.
- InstanceNorm mean needs no reduction: sum(z) over an instance equals
  512*256*x[0,0] exactly (DFT matrix rows sum to N*delta_0), so the
  mean is the DC element of the input.
- The variance reduction: row-wise sumsq partials on ACT, then one
  GpSimd partition_all_reduce yields partition-replicated stats so the
  normalize scalars need no broadcast.
- Matmul operands are fp16 (PSUM accumulation stays fp32): same 1
  cycle/row as float32r but the weight-load path is 2x faster, which
  otherwise paces the back-to-back matmul stream.
- Per-(b,c) stats/normalize chains are emitted one iteration behind the
  GEMM stream so the TensorEngine never waits on them.

kernel(**inputs) takes the FULL inputs and returns the FULL output.
"""
import sys

sys.path.insert(0, "/opt/trn_rl_repo")

import numpy as np

import concourse.bass as bass  # noqa: F401
import concourse.tile as tile
from concourse import bacc, bass_isa, mybir
from concourse.bass_utils import run_bass_kernel_spmd

B, C, R, D = 16, 16, 512, 256
NCORES = 8
BS = B // NCORES  # batches per core
EPS = 1e-5
N_NORM = R * D
F32 = mybir.dt.float32
F32R = mybir.dt.float32r
F16 = mybir.dt.float16
MULT = mybir.AluOpType.mult
ADD = mybir.AluOpType.add
SUB = mybir.AluOpType.subtract
COPY = mybir.ActivationFunctionType.Copy
SQRT = mybir.ActivationFunctionType.Sqrt
SQUARE = mybir.ActivationFunctionType.Square
X_AXIS = mybir.AxisListType.X


def build():
    nc = bacc.Bacc(None, target_bir_lowering=False)

    xr_d = nc.dram_tensor("x_real", [BS, C, R, D], F16, kind="ExternalInput")
    xi_d = nc.dram_tensor("x_imag", [BS, C, R, D], F16, kind="ExternalInput")
    xpi_d = nc.dram_tensor("x_pi", [BS, C, R, D], F16, kind="ExternalInput")
    wr512_d = nc.dram_tensor("Wr512", [512, 512], F16, kind="ExternalInput")
    wi512_d = nc.dram_tensor("Wi512", [512, 512], F16, kind="ExternalInput")
    nwrpi512_d = nc.dram_tensor("nWrpi512", [512, 512], F16,
                                kind="ExternalInput")
    # catA = [Wr256 | Wi256], catB = [-Wi256 | Wr256]  (both [256, 512])
    w256a_d = nc.dram_tensor("W256catA", [256, 512], F16, kind="ExternalInput")
    w256b_d = nc.dram_tensor("W256catB", [256, 512], F16, kind="ExternalInput")
    out_d = nc.dram_tensor("out", [BS, 2 * C, R, D], F32, kind="ExternalOutput")

    with tile.TileContext(nc) as tc:
        with tc.tile_pool(name="wpool", bufs=1) as wpool, \
             tc.tile_pool(name="xpool", bufs=4) as xpool, \
             tc.tile_pool(name="ypool", bufs=3) as ypool, \
             tc.tile_pool(name="zpool", bufs=4) as zpool, \
             tc.tile_pool(name="stpool", bufs=6) as stpool, \
             tc.tile_pool(name="sqpool", bufs=4) as sqpool, \
             tc.tile_pool(name="drpool", bufs=8, space="DRAM") as drpool, \
             tc.tile_pool(name="pspool", bufs=1, space="PSUM") as pspool:

            # --- weights, resident for the whole kernel ---
            # Spread across rings so the first matmuls aren't queued behind
            # 5MB of weights: wr/nwrpi on the ACT HWDGE ring, wi + w256 on
            # the SWDGE ring (x tiles own the SP ring).
            w512 = {}
            w256 = {}
            for nm, dram, shape, eng, store in (
                    ("wr", wr512_d, [128, 4, 512], nc.scalar, w512),
                    ("wi", wi512_d, [128, 4, 512], nc.gpsimd, w512),
                    ("nwrpi", nwrpi512_d, [128, 4, 512], nc.scalar, w512),
                    ("a", w256a_d, [128, 2, 512], nc.gpsimd, w256),
                    ("b", w256b_d, [128, 2, 512], nc.gpsimd, w256)):
                t = wpool.tile(shape, F16, name=f"w_{nm}")
                eng.dma_start(
                    out=t,
                    in_=dram[:].rearrange("(k p) n -> p k n", p=128))
                store[nm] = t
            eps128 = wpool.tile([128, 1], F32, name="eps128")
            nc.vector.memset(eps128, EPS)

            def emit_compute(b, c):
                """GEMM stream for one (b, c): loads, stage 1, stage 2,
                z PSUM->SBUF copies + sumsq partials. Returns state for the
                deferred stats/normalize pass."""
                xr = xpool.tile([128, 4, 256], F16, name="xr", tag="xr")
                nc.sync.dma_start(
                    out=xr,
                    in_=xr_d[b, c].rearrange("(k p) d -> p k d", p=128))
                xi = xpool.tile([128, 4, 256], F16, name="xi", tag="xi")
                nc.sync.dma_start(
                    out=xi,
                    in_=xi_d[b, c].rearrange("(k p) d -> p k d", p=128))

                xpi = xpool.tile([128, 4, 256], F16, name="xpi", tag="xpi")
                nc.sync.dma_start(
                    out=xpi,
                    in_=xpi_d[b, c].rearrange("(k p) d -> p k d", p=128))

                # --- stage 1 (Karatsuba): yT = (W512 @ x)^T ---
                yT = {}
                asbs = {}
                psB = {}
                for m in range(2):
                    pA = pspool.tile([128, 512], F32, name="ps1a",
                                     tag="ps1a", bufs=2)
                    pB = pspool.tile([128, 512], F32, name="ps1b",
                                     tag="ps1b", bufs=2)
                    psB[m] = pB
                    for k in range(4):
                        nc.tensor.matmul(
                            out=pA, lhsT=xr[:, k, m * 128:(m + 1) * 128],
                            rhs=w512["wr"][:, k, :],
                            start=(k == 0), stop=(k == 3))
                    for k in range(4):
                        nc.tensor.matmul(
                            out=pB, lhsT=xi[:, k, m * 128:(m + 1) * 128],
                            rhs=w512["wi"][:, k, :],
                            start=(k == 0), stop=(k == 3))
                    asb = ypool.tile([128, 512], F32, name=f"asb{m}",
                                     tag=f"asb{m}")
                    nc.vector.tensor_copy(out=asb, in_=pA)
                    asbs[m] = asb
                    bsb = ypool.tile([128, 512], F32, name=f"bsb{m}",
                                     tag=f"bsb{m}")
                    nc.vector.tensor_copy(out=bsb, in_=pB)
                    yt = ypool.tile([128, 512], F16, name=f"yT_r{m}",
                                    tag=f"yT_r{m}")
                    nc.vector.tensor_sub(out=yt, in0=asb, in1=bsb)
                    yT[("r", m)] = yt
                for m in range(2):
                    pB = psB[m]
                    for k in range(4):
                        nc.tensor.matmul(
                            out=pB, lhsT=xpi[:, k, m * 128:(m + 1) * 128],
                            rhs=w512["nwrpi"][:, k, :],
                            start=False, stop=(k == 3))
                    yt = ypool.tile([128, 512], F16, name=f"yT_i{m}",
                                    tag=f"yT_i{m}")
                    # yiT = (-1)*(M2 - M3) - M1
                    nc.vector.scalar_tensor_tensor(
                        out=yt, in0=pB, scalar=-1.0, in1=asbs[m],
                        op0=MULT, op1=SUB)
                    yT[("i", m)] = yt

                # --- stage 2: [zr | zi] = y @ [catA ; catB] ---
                # partials cols: 0-3 q_r (per m2), 4-7 q_i, 8-9 DC mean
                # (mean of the instance == DC input element, exactly)
                partials = stpool.tile([128, 10], F32, name="partials",
                                       tag="partials")
                nc.vector.memset(partials[:, 8:10], 0.0)
                nc.vector.tensor_copy(out=partials[0:1, 8:9],
                                      in_=xr[0:1, 0, 0:1])
                nc.vector.tensor_copy(out=partials[0:1, 9:10],
                                      in_=xi[0:1, 0, 0:1])
                z_r = zpool.tile([128, 4, 256], F32, name="z_r", tag="z_r")
                z_i = zpool.tile([128, 4, 256], F32, name="z_i", tag="z_i")
                zt = {"r": z_r, "i": z_i}
                for m2 in range(4):
                    ps2 = pspool.tile([128, 512], F32, name="ps2",
                                      tag="ps2", bufs=4)
                    n = 0
                    for src_comp, w in (("r", w256["a"]), ("i", w256["b"])):
                        for k2 in range(2):
                            nc.tensor.matmul(
                                out=ps2,
                                lhsT=yT[(src_comp, k2)][
                                    :, m2 * 128:(m2 + 1) * 128],
                                rhs=w[:, k2, :],
                                start=(n == 0), stop=(n == 3))
                            n += 1
                    for ci, comp in enumerate(("r", "i")):
                        half = ps2[:, ci * 256:(ci + 1) * 256]
                        col = ci * 4 + m2
                        # PSUM -> SBUF copy on ACT
                        nc.scalar.copy(out=zt[comp][:, m2, :], in_=half)
                        # row-wise sumsq on ACT (reads PSUM in parallel)
                        sq = sqpool.tile([128, 256], F32, name="sq", tag="sq")
                        nc.scalar.activation(
                            out=sq, in_=half, func=SQUARE,
                            accum_out=partials[:, col:col + 1])
                return dict(b=b, c=c, partials=partials, zt=zt)

            def emit_stats(st):
                """Deferred per-(b,c): cross-partition sumsq reduce (GpSimd,
                replicated to all partitions), stats math at [128,2], then
                normalize + store. No PE or DMA involvement."""
                b, c = st["b"], st["c"]
                partials, zt = st["partials"], st["zt"]
                allred = stpool.tile([128, 10], F32, name="allred",
                                     tag="allred")
                nc.gpsimd.partition_all_reduce(
                    allred, partials, channels=128,
                    reduce_op=bass_isa.ReduceOp.add)
                q2 = stpool.tile([128, 2], F32, name="q2", tag="q2")
                nc.vector.tensor_reduce(
                    out=q2,
                    in_=allred[:, 0:8].rearrange("p (g m) -> p g m", m=4),
                    axis=X_AXIS, op=ADD)
                # var = E[z^2] - mean^2 ; istd = 1/sqrt(var + eps)
                e2 = stpool.tile([128, 2], F32, name="e2", tag="e2")
                nc.vector.tensor_scalar_mul(out=e2, in0=q2,
                                            scalar1=1.0 / N_NORM)
                mean2 = allred[:, 8:10]
                msq = stpool.tile([128, 2], F32, name="msq", tag="msq")
                nc.vector.tensor_mul(out=msq, in0=mean2, in1=mean2)
                var2 = stpool.tile([128, 2], F32, name="var2", tag="var2")
                nc.vector.tensor_sub(out=var2, in0=e2, in1=msq)
                std2 = stpool.tile([128, 2], F32, name="std2", tag="std2")
                nc.scalar.activation(out=std2, in_=var2, func=SQRT,
                                     bias=eps128, scale=1.0)
                istd = stpool.tile([128, 2], F32, name="istd", tag="istd")
                nc.vector.reciprocal(out=istd, in_=std2)
                mb = stpool.tile([128, 2], F32, name="mb", tag="mb")
                nc.vector.tensor_mul(out=mb, in0=mean2, in1=istd)
                # normalize in place and store
                for ci, comp in enumerate(("r", "i")):
                    z = zt[comp]
                    nc.vector.tensor_scalar(
                        out=z, in0=z,
                        scalar1=istd[:, ci:ci + 1],
                        scalar2=mb[:, ci:ci + 1],
                        op0=MULT, op1=SUB)
                    ch = c if comp == "r" else C + c
                    nc.sync.dma_start(
                        out=out_d[b, ch].rearrange("(k p) d -> p k d", p=128),
                        in_=z)

            prev = None
            for b in range(BS):
                for c in range(C):
                    st = emit_compute(b, c)
                    if prev is not None:
                        emit_stats(prev)
                    prev = st
            emit_stats(prev)

    nc.finalize()
    return nc


_NC_CACHE = None


def _get_nc():
    global _NC_CACHE
    if _NC_CACHE is None:
        _NC_CACHE = build()
    return _NC_CACHE


def make_in_maps(inputs):
    xr = np.ascontiguousarray(np.asarray(inputs["x_real"], dtype=np.float32).astype(np.float16))
    xi = np.ascontiguousarray(np.asarray(inputs["x_imag"], dtype=np.float32).astype(np.float16))
    xpi = np.ascontiguousarray(
        (np.asarray(inputs["x_real"], dtype=np.float32)
         + np.asarray(inputs["x_imag"], dtype=np.float32)).astype(np.float16))
    wr512 = np.ascontiguousarray(np.asarray(inputs["Wr512"], dtype=np.float32))
    wi512 = np.ascontiguousarray(np.asarray(inputs["Wi512"], dtype=np.float32))
    wr256 = np.ascontiguousarray(np.asarray(inputs["Wr256"], dtype=np.float32))
    wi256 = np.ascontiguousarray(np.asarray(inputs["Wi256"], dtype=np.float32))
    nwrpi512 = np.ascontiguousarray((-(wr512 + wi512)).astype(np.float16))
    w256a = np.ascontiguousarray(np.concatenate([wr256, wi256], axis=1).astype(np.float16))
    w256b = np.ascontiguousarray(np.concatenate([-wi256, wr256], axis=1).astype(np.float16))
    in_maps = []
    for i in range(NCORES):
        in_maps.append({
            "x_real": np.ascontiguousarray(xr[i * BS:(i + 1) * BS]),
            "x_imag": np.ascontiguousarray(xi[i * BS:(i + 1) * BS]),
            "x_pi": np.ascontiguousarray(xpi[i * BS:(i + 1) * BS]),
            "Wr512": wr512.astype(np.float16), "Wi512": wi512.astype(np.float16), "nWrpi512": nwrpi512,
            "W256catA": w256a, "W256catB": w256b,
        })
    return in_maps


def run(inputs, trace=False):
    nc = _get_nc()
    in_maps = make_in_maps(inputs)
    try:
        res = run_bass_kernel_spmd(nc, in_maps, list(range(NCORES)),
                                   trace=trace)
    except Exception:
        # transient device wedge (NRT_EXEC_UNIT_UNRECOVERABLE): retry once
        res = run_bass_kernel_spmd(nc, in_maps, list(range(NCORES)),
                                   trace=trace)
    out = np.concatenate([res.results[i]["out"] for i in range(NCORES)],
                         axis=0)
    return out, res


def kernel(**inputs):
    out, _ = run(inputs, trace=False)
    return out


if __name__ == "__main__":
    rng = np.random.default_rng(0)
    ins = {
        "x_real": rng.standard_normal((B, C, R, D)).astype(np.float32),
        "x_imag": rng.standard_normal((B, C, R, D)).astype(np.float32),
    }
    n = np.arange(512)
    W = np.exp(-2j * np.pi * np.outer(n, n) / 512).astype(np.complex64)
    ins["Wr512"], ins["Wi512"] = W.real.copy(), W.imag.copy()
    n = np.arange(256)
    W = np.exp(-2j * np.pi * np.outer(n, n) / 256).astype(np.complex64)
    ins["Wr256"], ins["Wi256"] = W.real.copy(), W.imag.copy()
    out = kernel(**ins)
    print("out", out.shape, out.dtype, float(np.abs(out).mean()))
